# revision 24
# baseline (speedup 1.0000x reference)
"""Distributed single-head attention on 8 TRN2 NeuronCores.

softmax(Q @ K.T / sqrt(128)) @ V  with Q,K,V: [8192, 128] fp32.

Strategy: query-parallel. Q rows are sharded 8 ways (1024 queries/core);
K and V are replicated (no collectives). Each core runs flash-attention
style in the "S^T" layout (partitions = keys) so the PV matmul needs no
transpose of the probability tiles:

  S^T[k, q] = (K^T tile).T @ Q^T        (K^T tile stationary, Q^T moving)
  P^T       = exp(S^T / sqrt(128))      (ACT, fused scale; no max-sub
                                         needed: |scores| <= ~6 in fp32)
  O^T[d, q] += (V_tile).T @ P^T
  l[q]      = colsum(sum_t P^T_t)       (bf16 running accum on DVE)
  O         = transpose(O^T) * (1/l)

Engine floors per core: PE matmuls ~55us bf16, ACT exp 8.4M elements at
1 elem/cycle/partition @1.2GHz = ~55us + 352 cycles/instruction fixed
cost. The design keeps both engines lean:
  - ACT exp runs as 32x [128, 2048] instructions over a 6-bank PSUM
    ring of [128,512] S^T chunks (the other 2 banks hold the O^T
    accumulator). Odd-numbered windows wrap the ring via a
    negative-stride AP.
  - K^T and Q^T come from the DMA XBAR transpose, not PE transposes;
    the epilogue transposes (l and O^T) also go through the XBAR, so
    the PE runs matmuls only.
  - fp32->bf16 casts of Q/K/V run on the otherwise-idle gpsimd.
  - l accumulates into a 2-lane bf16 accumulator, one DVE add per exp.
"""

import sys

try:
    import concourse  # noqa: F401
except ImportError:  # grading container fallback
    sys.path.insert(0, "/opt/trn_rl_repo")

import numpy as np

import concourse.tile as tile
from concourse import bacc, mybir
from concourse.bass_utils import run_bass_kernel_spmd

N_CORES = 8
NQ, NK, D = 8192, 8192, 128
NQS = NQ // N_CORES          # queries per core
KT_TILES = NK // 128         # 64 key tiles of 128
SCALE = 1.0 / np.sqrt(np.float32(D))
NGROUPS = KT_TILES // 2      # 32 exp groups of 2 key tiles

F32 = mybir.dt.float32
BF16 = mybir.dt.bfloat16
EXP = mybir.ActivationFunctionType.Exp

_COMPILED = None


def _build():
    nc = bacc.Bacc(
        "TRN2", target_bir_lowering=False, debug=False, num_devices=N_CORES
    )
    q_d = nc.dram_tensor("Q", [NQS, D], F32, kind="ExternalInput").ap()
    k_d = nc.dram_tensor("K", [NK, D], F32, kind="ExternalInput").ap()
    v_d = nc.dram_tensor("V", [NK, D], F32, kind="ExternalInput").ap()
    o_d = nc.dram_tensor("out", [NQS, D], F32, kind="ExternalOutput").ap()

    # tile views: row = a*128 + p
    q_r = q_d.rearrange("(a p) d -> p a d", p=128)   # [128, 8, 128]
    k_r = k_d.rearrange("(a p) d -> p a d", p=128)   # [128, 64, 128]
    v_r = v_d.rearrange("(a p) d -> p a d", p=128)
    o_r = o_d.rearrange("(a p) d -> p a d", p=128)   # [128, 8, 128]

    with tile.TileContext(nc) as tc:
        with (
            tc.tile_pool(name="persist", bufs=1) as persist,
            tc.tile_pool(name="kb", bufs=4) as kb_pool,
            tc.tile_pool(name="ktg", bufs=5) as ktg_pool,
            tc.tile_pool(name="pt", bufs=3) as pt_pool,
            tc.tile_pool(name="psum_s", bufs=1, space="PSUM") as psum_s,
            tc.tile_pool(name="psum_o", bufs=1, space="PSUM") as psum_o,
        ):
            qt = persist.tile([128, 8, 128], BF16)     # Q^T  [d, a, q]
            acc2 = persist.tile([128, 2, 1024], BF16)  # P^T 2-lane accum
            acct = persist.tile([128, 16, 128], BF16)  # acc2 transposed
            lq = persist.tile([128, NQS // 128], F32)  # l in [q,1] layout
            rlq = persist.tile([128, NQS // 128], F32)  # 1/l
            ob = persist.tile([128, 1024], BF16)       # O^T in bf16
            ot = persist.tile([128, 8, 128], BF16)     # O transposed
            out_sb = persist.tile([128, NQS // 128, D], F32)

            # --- prologue: few big fp32 loads upfront (chunked so early
            # tiles land early), casts paced with the loop: K on DVE,
            # V on the otherwise-idle gpsimd. The sync queue carries the
            # K^T XBAR transposes; nothing blocks a queue head waiting
            # for a cast.
            NKG = KT_TILES // 4
            NVS = KT_TILES // 8
            kst = persist.tile([128, 64, 128], F32)   # K fp32 staging
            vst = persist.tile([128, 64, 128], F32)   # V fp32 staging
            qst = persist.tile([128, 8, 128], F32)
            # Q rides alone on the scalar queue's DMA stream so it lands
            # first; all K/V loads stream on the sync queue in need
            # order. Transposes go on the scalar queue: per-queue DMA is
            # processed in order, so they must not sit behind the bulk
            # loads.
            nc.scalar.dma_start(out=qst, in_=q_r)
            for lo, hi in ((0, 8), (8, 24), (24, 40), (40, 64)):
                nc.sync.dma_start(out=kst[:, lo:hi, :], in_=k_r[:, lo:hi, :])
                nc.sync.dma_start(out=vst[:, lo:hi, :], in_=v_r[:, lo:hi, :])

            qb = persist.tile([128, 8, 128], BF16)
            nc.vector.tensor_copy(out=qb, in_=qst)
            nc.scalar.dma_start_transpose(out=qt, in_=qb)
            nc.gpsimd.memset(acc2, 0.0)

            vsbs = [
                persist.tile([128, 8, 128], BF16, name=f"vsb{s}")
                for s in range(NVS)
            ]
            ktgs, pts = {}, {}
            k_transposed = [0]  # next K group to cast + transpose
            v_cast = [0]        # next V stage to cast

            def ensure_k(upto):  # make ktg groups [0, upto) available
                while k_transposed[0] < min(upto, NKG):
                    g = k_transposed[0]
                    kb = kb_pool.tile([128, 4, 128], BF16, tag="kb")
                    nc.vector.tensor_copy(out=kb, in_=kst[:, 4 * g : 4 * g + 4, :])
                    ktg = ktg_pool.tile([128, 4, 128], BF16, tag="ktg")
                    nc.scalar.dma_start_transpose(out=ktg, in_=kb)
                    ktgs[g] = ktg
                    k_transposed[0] += 1

            def ensure_v(upto):  # make vsb stages [0, upto) available
                while v_cast[0] < min(upto, NVS):
                    s = v_cast[0]
                    nc.gpsimd.tensor_copy(
                        out=vsbs[s], in_=vst[:, 8 * s : 8 * s + 8, :]
                    )
                    v_cast[0] += 1

            # prime the pipelines
            ensure_k(2)
            ensure_v(2)

            # S^T ring: 6 banks of [128, 512] chunks; chunk (2t+c) % 6
            sring = psum_s.tile([128, 6, 512], F32)
            sflat = sring.rearrange("p a f -> p (a f)")  # [128, 3072]
            po = psum_o.tile([128, NQS], F32)  # O^T accum, both chunks

            def s_group(m):  # S^T matmuls + exp + l-accum, tiles 2m, 2m+1
                ensure_k((2 * m + 1) // 4 + 3)
                for i in range(2):
                    t = 2 * m + i
                    g4, a = divmod(t, 4)
                    lhs = ktgs[g4][:, a, :]
                    for c in range(2):
                        nc.tensor.matmul(
                            sring[:, (2 * t + c) % 6, :],
                            lhs,
                            qt[:, 4 * c : 4 * c + 4, :],
                            start=True,
                            stop=True,
                        )
                    if a == 3:
                        ktgs.pop(g4)
                # exp window: chunks 4m..4m+3 (mod 6). Window starts cycle
                # 0, 4, 2 (period 3). All APs are flat 2D contiguous
                # slices so subtile dependency tracking stays precise
                # (3D/4D strided APs fall back to whole-tile deps and
                # serialize the PE behind every exp). The start-4 window
                # wraps the ring: two cleanly-sliced instructions.
                r = m % 3
                pt = pt_pool.tile([128, 2, 1024], BF16, tag="pt")
                ptf = pt.rearrange("p i f -> p (i f)")  # [128, 2048]
                if r == 0:
                    nc.scalar.activation(
                        ptf, sflat[:, 0:2048], EXP, scale=float(SCALE)
                    )
                elif r == 2:
                    nc.scalar.activation(
                        ptf, sflat[:, 1024:3072], EXP, scale=float(SCALE)
                    )
                else:
                    nc.scalar.activation(
                        ptf[:, 0:1024], sflat[:, 2048:3072], EXP,
                        scale=float(SCALE),
                    )
                    nc.scalar.activation(
                        ptf[:, 1024:2048], sflat[:, 0:1024], EXP,
                        scale=float(SCALE),
                    )
                # flat 2D views: 3D APs miss the DVE 2x packed mode
                nc.vector.tensor_add(
                    acc2.rearrange("p i f -> p (i f)"),
                    acc2.rearrange("p i f -> p (i f)"),
                    pt.rearrange("p i f -> p (i f)"),
                )
                pts[m] = pt

            def pv_group(m):  # O^T accumulation for tiles 2m, 2m+1
                ensure_v((2 * m + 1) // 8 + 3)
                pt = pts.pop(m)
                for i in range(2):
                    t = 2 * m + i
                    vsb = vsbs[t // 8]
                    for c in range(2):
                        nc.tensor.matmul(
                            po[:, 512 * c : 512 * (c + 1)],
                            vsb[:, t % 8, :],
                            pt[:, i, 512 * c : 512 * (c + 1)],
                            start=(t == 0),
                            stop=(t == KT_TILES - 1),
                        )

            # --- main pipeline: PV trails S/exp by one group
            for m in range(NGROUPS + 1):
                if m < NGROUPS:
                    s_group(m)
                if m >= 1:
                    pv_group(m - 1)

            # --- epilogue (XBAR transposes; PE stays matmul-only) ---
            # l: transpose both accumulator lanes, reduce over keys,
            # sum the lane halves
            nc.scalar.dma_start_transpose(
                out=acct, in_=acc2.rearrange("p i f -> p (i f)")
            )
            lq16 = persist.tile([128, 16], F32)
            nc.vector.tensor_reduce(
                lq16, acct, axis=mybir.AxisListType.X, op=mybir.AluOpType.add
            )
            nc.vector.tensor_add(lq, lq16[:, 0:8], lq16[:, 8:16])
            nc.vector.reciprocal(rlq, lq)
            # O: cast O^T to bf16, transpose, scale rows by 1/l
            nc.vector.tensor_copy(out=ob, in_=po)
            nc.scalar.dma_start_transpose(out=ot, in_=ob)
            for a in range(8):
                nc.vector.tensor_scalar_mul(
                    out_sb[:, a, :], ot[:, a, :], rlq[:, a : a + 1]
                )
            nc.sync.dma_start(out=o_r, in_=out_sb)

    nc.compile()
    return nc


def _get_compiled():
    global _COMPILED
    if _COMPILED is None:
        _COMPILED = _build()
    return _COMPILED


def kernel(Q, K, V):
    assert Q.shape == (NQ, D) and K.shape == (NK, D) and V.shape == (NK, D), (
        Q.shape, K.shape, V.shape
    )
    Q = np.ascontiguousarray(np.asarray(Q, dtype=np.float32))
    K = np.ascontiguousarray(np.asarray(K, dtype=np.float32))
    V = np.ascontiguousarray(np.asarray(V, dtype=np.float32))
    nc = _get_compiled()
    in_maps = [
        {"Q": Q[i * NQS : (i + 1) * NQS], "K": K, "V": V} for i in range(N_CORES)
    ]
    res = run_bass_kernel_spmd(nc, in_maps, list(range(N_CORES)))
    out = np.concatenate([r["out"] for r in res.results], axis=0)
    return out.astype(np.float32)


# revision 26
# speedup vs baseline: 1.0289x; 1.0289x over previous
"""Distributed single-head attention on 8 TRN2 NeuronCores.

softmax(Q @ K.T / sqrt(128)) @ V  with Q,K,V: [8192, 128] fp32.

Strategy: query-parallel. Q rows are sharded 8 ways (1024 queries/core);
K and V are replicated (no collectives). Each core runs flash-attention
style in the "S^T" layout (partitions = keys) so the PV matmul needs no
transpose of the probability tiles:

  S^T[k, q] = (K^T tile).T @ Q^T        (K^T tile stationary, Q^T moving)
  P^T       = exp(S^T / sqrt(128))      (ACT, fused scale; no max-sub
                                         needed: |scores| <= ~6 in fp32)
  O^T[d, q] += (V_tile).T @ P^T
  l[q]      = colsum(sum P^T)           (bf16 running accum on DVE)
  O         = transpose(O^T) * (1/l)

The kernel is ACT(exp)-bound: 8.4M elements at 1 elem/cycle/partition
@1.2GHz plus ~350 cycles fixed cost per ACTIVATE. Design:
  - S^T lives in a 6-bank PSUM ring of [128,512] chunks (chunk
    (2t+c) mod 6); exp runs as 43x [128,1536] instructions whose
    3-chunk windows alternate between ring halves, so matmul
    production of the next window never waits on the current exp
    (true double buffering; the other 2 banks hold the O^T
    accumulator). All exp APs are flat 2D contiguous slices: 3D+
    strided APs defeat subtile dependency tracking and serialize.
  - K^T and Q^T come from the DMA XBAR transpose (sync/scalar
    queues carry transposes only; bulk fp32 loads stream on the sync
    queue in need-order chunks - per-queue DMA is processed in
    order).
  - fp32->bf16: K/Q on DVE, V on the otherwise-idle gpsimd.
  - l accumulates into a 6-lane bf16 accumulator (one DVE add per
    exp window), folded in the epilogue.
  - a dummy 1-element ACTIVATE at t=0 hoists the ~2.7us exp table
    load off the critical path.
"""

import sys

try:
    import concourse  # noqa: F401
except ImportError:  # grading container fallback
    sys.path.insert(0, "/opt/trn_rl_repo")

import numpy as np

import concourse.tile as tile
from concourse import bacc, mybir
from concourse.bass_utils import run_bass_kernel_spmd

N_CORES = 8
NQ, NK, D = 8192, 8192, 128
NQS = NQ // N_CORES          # queries per core
KT_TILES = NK // 128         # 64 key tiles of 128
NCHUNKS = 2 * KT_TILES       # 128 S^T chunks of [128, 512]
SCALE = 1.0 / np.sqrt(np.float32(D))

F32 = mybir.dt.float32
BF16 = mybir.dt.bfloat16
EXP = mybir.ActivationFunctionType.Exp

# exp windows over the 128 chunks: ceil(128/3) windows of 3 chunks
# (the last window covers 2). Window w reads chunks 3w..3w+2, i.e.
# ring offset alternates 0 / 3.
WINDOWS = [(3 * w, min(3, NCHUNKS - 3 * w)) for w in range((NCHUNKS + 2) // 3)]

_COMPILED = None


def _build():
    nc = bacc.Bacc(
        "TRN2", target_bir_lowering=False, debug=False, num_devices=N_CORES
    )
    q_d = nc.dram_tensor("Q", [NQS, D], F32, kind="ExternalInput").ap()
    k_d = nc.dram_tensor("K", [NK, D], F32, kind="ExternalInput").ap()
    v_d = nc.dram_tensor("V", [NK, D], F32, kind="ExternalInput").ap()
    o_d = nc.dram_tensor("out", [NQS, D], F32, kind="ExternalOutput").ap()

    # tile views: row = a*128 + p
    q_r = q_d.rearrange("(a p) d -> p a d", p=128)   # [128, 8, 128]
    k_r = k_d.rearrange("(a p) d -> p a d", p=128)   # [128, 64, 128]
    v_r = v_d.rearrange("(a p) d -> p a d", p=128)
    o_r = o_d.rearrange("(a p) d -> p a d", p=128)   # [128, 8, 128]

    with tile.TileContext(nc) as tc:
        with (
            tc.tile_pool(name="persist", bufs=1) as persist,
            tc.tile_pool(name="kb", bufs=4) as kb_pool,
            tc.tile_pool(name="ktg", bufs=5) as ktg_pool,
            tc.tile_pool(name="pt", bufs=4) as pt_pool,
            tc.tile_pool(name="psum_s", bufs=1, space="PSUM") as psum_s,
            tc.tile_pool(name="psum_o", bufs=1, space="PSUM") as psum_o,
        ):
            qt = persist.tile([128, 8, 128], BF16)     # Q^T  [d, a, q]
            acc6 = persist.tile([128, 6, 512], BF16)   # P^T 6-lane accum
            acc6f = acc6.rearrange("p i f -> p (i f)")
            lq = persist.tile([128, NQS // 128], F32)  # l in [q,1] layout
            rlq = persist.tile([128, NQS // 128], F32)  # 1/l
            ob = persist.tile([128, 1024], BF16)       # O^T in bf16
            ot = persist.tile([128, 8, 128], BF16)     # O transposed
            out_sb = persist.tile([128, NQS // 128, D], F32)
            scr = persist.tile([128, 8], BF16)         # table-load dummy

            # warm the exp table set immediately (off the critical path)
            nc.scalar.activation(scr, scr, EXP)

            # bulk fp32 loads, need-order chunks, sync queue only
            kst = persist.tile([128, 64, 128], F32)   # K fp32 staging
            vst = persist.tile([128, 64, 128], F32)   # V fp32 staging
            qst = persist.tile([128, 8, 128], F32)
            nc.scalar.dma_start(out=qst, in_=q_r)
            nc.sync.dma_start(out=kst[:, 0:4, :], in_=k_r[:, 0:4, :])
            nc.sync.dma_start(out=kst[:, 4:12, :], in_=k_r[:, 4:12, :])
            nc.sync.dma_start(out=vst[:, 0:8, :], in_=v_r[:, 0:8, :])
            nc.sync.dma_start(out=kst[:, 12:24, :], in_=k_r[:, 12:24, :])
            nc.sync.dma_start(out=vst[:, 8:24, :], in_=v_r[:, 8:24, :])
            nc.sync.dma_start(out=kst[:, 24:40, :], in_=k_r[:, 24:40, :])
            nc.sync.dma_start(out=vst[:, 24:40, :], in_=v_r[:, 24:40, :])
            nc.sync.dma_start(out=kst[:, 40:64, :], in_=k_r[:, 40:64, :])
            nc.sync.dma_start(out=vst[:, 40:64, :], in_=v_r[:, 40:64, :])

            qb = persist.tile([128, 8, 128], BF16)
            nc.vector.tensor_copy(out=qb, in_=qst)
            nc.scalar.dma_start_transpose(out=qt, in_=qb)
            nc.gpsimd.memset(acc6, 0.0)

            NKG = KT_TILES // 4
            NVS = KT_TILES // 8
            vsbs = [
                persist.tile([128, 8, 128], BF16, name=f"vsb{s}")
                for s in range(NVS)
            ]
            ktgs, pts = {}, {}
            k_transposed = [0]  # next K group to cast + transpose
            v_cast = [0]        # next V stage to cast

            def ensure_k(upto):  # make ktg groups [0, upto) available
                while k_transposed[0] < min(upto, NKG):
                    g = k_transposed[0]
                    kb = kb_pool.tile([128, 4, 128], BF16, tag="kb")
                    nc.vector.tensor_copy(
                        out=kb, in_=kst[:, 4 * g : 4 * g + 4, :]
                    )
                    ktg = ktg_pool.tile([128, 4, 128], BF16, tag="ktg")
                    nc.scalar.dma_start_transpose(out=ktg, in_=kb)
                    ktgs[g] = ktg
                    k_transposed[0] += 1

            def ensure_v(upto):  # make vsb stages [0, upto) available
                while v_cast[0] < min(upto, NVS):
                    s = v_cast[0]
                    nc.gpsimd.tensor_copy(
                        out=vsbs[s], in_=vst[:, 8 * s : 8 * s + 8, :]
                    )
                    v_cast[0] += 1

            # prime the pipelines
            ensure_k(2)
            ensure_v(2)

            # S^T ring: 6 banks of [128, 512] chunks; chunk (2t+c) % 6
            sring = psum_s.tile([128, 6, 512], F32)
            sflat = sring.rearrange("p a f -> p (a f)")  # [128, 3072]
            po = psum_o.tile([128, NQS], F32)  # O^T accum, both chunks

            s_chunk = [0]   # next S^T chunk to produce (2t+c)

            def produce_chunks(upto):  # S^T matmuls for chunks [0, upto)
                while s_chunk[0] < min(upto, NCHUNKS):
                    k = s_chunk[0]
                    t, c = divmod(k, 2)
                    g4, a = divmod(t, 4)
                    ensure_k(g4 + 3)
                    nc.tensor.matmul(
                        sring[:, k % 6, :],
                        ktgs[g4][:, a, :],
                        qt[:, 4 * c : 4 * c + 4, :],
                        start=True,
                        stop=True,
                    )
                    if a == 3 and c == 1:
                        ktgs.pop(g4)
                    s_chunk[0] += 1

            def exp_window(w):  # exp over chunks 3w..3w+2, l-accum
                start, n = WINDOWS[w]
                produce_chunks(start + n)
                lo = (start % 6) * 512
                pt = pt_pool.tile([128, 3, 512], BF16, tag="pt")
                ptf = pt.rearrange("p i f -> p (i f)")
                nc.scalar.activation(
                    ptf[:, : 512 * n], sflat[:, lo : lo + 512 * n], EXP,
                    scale=float(SCALE),
                )
                nc.vector.tensor_add(
                    acc6f[:, lo : lo + 512 * n],
                    acc6f[:, lo : lo + 512 * n],
                    ptf[:, : 512 * n],
                )
                pts[w] = pt

            pv_chunk = [0]  # next chunk to consume in PV

            def pv_consume(upto):  # PV matmuls for chunks [0, upto)
                while pv_chunk[0] < min(upto, NCHUNKS):
                    k = pv_chunk[0]
                    t, c = divmod(k, 2)
                    ensure_v(t // 8 + 2)
                    w, i = divmod(k, 3)
                    nc.tensor.matmul(
                        po[:, 512 * c : 512 * (c + 1)],
                        vsbs[t // 8][:, t % 8, :],
                        pts[w][:, i, :],
                        start=(t == 0),
                        stop=(t == KT_TILES - 1),
                    )
                    if i == 2 or k == NCHUNKS - 1:
                        pts.pop(w - 1, None)  # w-1 fully consumed earlier
                    pv_chunk[0] += 1

            # --- main pipeline: PV trails exp by one window
            NW = len(WINDOWS)
            for w in range(NW + 1):
                if w < NW:
                    exp_window(w)
                if w >= 1:
                    pv_consume(WINDOWS[w - 1][0] + WINDOWS[w - 1][1])

            # --- epilogue ---
            # l: fold 6 lanes -> [q, a] via XBAR transpose + reduces.
            # acct[qf, g, kf] = acc6f[kf, 128g + qf]; lane L = g//4 holds
            # q-half L%2, sub-block a4 = g%4.
            acct = persist.tile([128, 24, 128], BF16)
            nc.scalar.dma_start_transpose(out=acct, in_=acc6f)
            r24 = persist.tile([128, 24], F32)
            nc.vector.tensor_reduce(
                r24, acct, axis=mybir.AxisListType.X, op=mybir.AluOpType.add
            )
            r24v = r24.rearrange("p (L a) -> p L a", L=6)
            for c in range(2):
                nc.vector.tensor_reduce(
                    lq[:, 4 * c : 4 * c + 4],
                    r24v[:, c::2, :].rearrange("p L a -> p a L"),
                    axis=mybir.AxisListType.X,
                    op=mybir.AluOpType.add,
                )
            nc.vector.reciprocal(rlq, lq)
            # O: cast O^T to bf16, transpose, scale rows by 1/l
            nc.vector.tensor_copy(out=ob, in_=po)
            nc.sync.dma_start_transpose(out=ot, in_=ob)
            for a in range(8):
                nc.vector.tensor_scalar_mul(
                    out_sb[:, a, :], ot[:, a, :], rlq[:, a : a + 1]
                )
            nc.sync.dma_start(out=o_r, in_=out_sb)

    nc.compile()
    return nc


def _get_compiled():
    global _COMPILED
    if _COMPILED is None:
        _COMPILED = _build()
    return _COMPILED


def kernel(Q, K, V):
    assert Q.shape == (NQ, D) and K.shape == (NK, D) and V.shape == (NK, D), (
        Q.shape, K.shape, V.shape
    )
    Q = np.ascontiguousarray(np.asarray(Q, dtype=np.float32))
    K = np.ascontiguousarray(np.asarray(K, dtype=np.float32))
    V = np.ascontiguousarray(np.asarray(V, dtype=np.float32))
    nc = _get_compiled()
    in_maps = [
        {"Q": Q[i * NQS : (i + 1) * NQS], "K": K, "V": V} for i in range(N_CORES)
    ]
    res = run_bass_kernel_spmd(nc, in_maps, list(range(N_CORES)))
    out = np.concatenate([r["out"] for r in res.results], axis=0)
    return out.astype(np.float32)


# revision 28
# speedup vs baseline: 1.0513x; 1.0217x over previous
"""Distributed single-head attention on 8 TRN2 NeuronCores.

softmax(Q @ K.T / sqrt(128)) @ V  with Q,K,V: [8192, 128] fp32.

Strategy: query-parallel. Q rows are sharded 8 ways (1024 queries/core);
K and V are replicated (no collectives). Each core runs flash-attention
style in the "S^T" layout (partitions = keys) so the PV matmul needs no
transpose of the probability tiles:

  S^T[k, q] = (K^T tile).T @ Q^T        (K^T tile stationary, Q^T moving)
  P^T       = exp(S^T / sqrt(128))      (ACT, fused scale; no max-sub
                                         needed: |scores| <= ~6 in fp32)
  O^T[d, q] += (V_tile).T @ P^T
  l[q]      = colsum(sum P^T)           (bf16 running accum on DVE)
  O         = transpose(O^T) * (1/l)

The kernel is ACT(exp)-bound: 8.4M elements at 1 elem/cycle/partition
@1.2GHz plus ~350 cycles fixed cost per ACTIVATE. Design:
  - S^T lives in a 6-bank PSUM ring of [128,512] chunks (chunk
    (2t+c) mod 6); exp runs as 43x [128,1536] instructions whose
    3-chunk windows alternate between ring halves, so matmul
    production of the next window never waits on the current exp
    (true double buffering; the other 2 banks hold the O^T
    accumulator). All exp APs are flat 2D contiguous slices: 3D+
    strided APs defeat subtile dependency tracking and serialize.
  - K^T and Q^T come from the DMA XBAR transpose (sync/scalar
    queues carry transposes only; bulk fp32 loads stream on the sync
    queue in need-order chunks - per-queue DMA is processed in
    order).
  - fp32->bf16: K/Q on DVE, V on the otherwise-idle gpsimd.
  - l accumulates into a 6-lane bf16 accumulator (one DVE add per
    exp window), folded in the epilogue.
  - a dummy 1-element ACTIVATE at t=0 hoists the ~2.7us exp table
    load off the critical path.
"""

import sys

try:
    import concourse  # noqa: F401
except ImportError:  # grading container fallback
    sys.path.insert(0, "/opt/trn_rl_repo")

import numpy as np

import concourse.tile as tile
from concourse import bacc, mybir
from concourse.bass_utils import run_bass_kernel_spmd

N_CORES = 8
NQ, NK, D = 8192, 8192, 128
NQS = NQ // N_CORES          # queries per core
KT_TILES = NK // 128         # 64 key tiles of 128
NCHUNKS = 2 * KT_TILES       # 128 S^T chunks of [128, 512]
SCALE = 1.0 / np.sqrt(np.float32(D))

F32 = mybir.dt.float32
BF16 = mybir.dt.bfloat16
EXP = mybir.ActivationFunctionType.Exp

# exp windows over the 128 chunks: ceil(128/3) windows of 3 chunks
# (the last window covers 2). Window w reads chunks 3w..3w+2, i.e.
# ring offset alternates 0 / 3.
WINDOWS = [(3 * w, min(3, NCHUNKS - 3 * w)) for w in range((NCHUNKS + 2) // 3)]

_COMPILED = None


def _build():
    nc = bacc.Bacc(
        "TRN2", target_bir_lowering=False, debug=False, num_devices=N_CORES
    )
    q_d = nc.dram_tensor("Q", [NQS, D], F32, kind="ExternalInput").ap()
    k_d = nc.dram_tensor("K", [NK, D], F32, kind="ExternalInput").ap()
    v_d = nc.dram_tensor("V", [NK, D], F32, kind="ExternalInput").ap()
    o_d = nc.dram_tensor("out", [NQS, D], F32, kind="ExternalOutput").ap()

    # tile views: row = a*128 + p
    q_r = q_d.rearrange("(a p) d -> p a d", p=128)   # [128, 8, 128]
    k_r = k_d.rearrange("(a p) d -> p a d", p=128)   # [128, 64, 128]
    v_r = v_d.rearrange("(a p) d -> p a d", p=128)
    o_r = o_d.rearrange("(a p) d -> p a d", p=128)   # [128, 8, 128]

    with tile.TileContext(nc) as tc:
        with (
            tc.tile_pool(name="persist", bufs=1) as persist,
            tc.tile_pool(name="kb", bufs=4) as kb_pool,
            tc.tile_pool(name="ktg", bufs=5) as ktg_pool,
            tc.tile_pool(name="pt", bufs=4) as pt_pool,
            tc.tile_pool(name="psum_s", bufs=1, space="PSUM") as psum_s,
            tc.tile_pool(name="psum_o", bufs=1, space="PSUM") as psum_o,
        ):
            qt = persist.tile([128, 8, 128], BF16)     # Q^T  [d, a, q]
            acc6 = persist.tile([128, 6, 512], BF16)   # P^T 6-lane accum
            acc6f = acc6.rearrange("p i f -> p (i f)")
            lq = persist.tile([128, NQS // 128], F32)  # l in [q,1] layout
            rlq = persist.tile([128, NQS // 128], F32)  # 1/l
            ob = persist.tile([128, 1024], BF16)       # O^T in bf16
            ot = persist.tile([128, 8, 128], BF16)     # O transposed
            out_sb = persist.tile([128, NQS // 128, D], F32)
            scr = persist.tile([128, 8], BF16)         # table-load dummy

            # warm the exp table set immediately (off the critical path)
            nc.scalar.activation(scr, scr, EXP)

            # bulk fp32 loads, need-order chunks, sync queue only
            kst = persist.tile([128, 64, 128], F32)   # K fp32 staging
            vst = persist.tile([128, 64, 128], F32)   # V fp32 staging
            qst = persist.tile([128, 8, 128], F32)
            nc.scalar.dma_start(out=qst, in_=q_r)
            nc.sync.dma_start(out=kst[:, 0:4, :], in_=k_r[:, 0:4, :])
            nc.sync.dma_start(out=kst[:, 4:12, :], in_=k_r[:, 4:12, :])
            nc.sync.dma_start(out=vst[:, 0:8, :], in_=v_r[:, 0:8, :])
            nc.sync.dma_start(out=kst[:, 12:24, :], in_=k_r[:, 12:24, :])
            nc.sync.dma_start(out=vst[:, 8:24, :], in_=v_r[:, 8:24, :])
            nc.sync.dma_start(out=kst[:, 24:40, :], in_=k_r[:, 24:40, :])
            nc.sync.dma_start(out=vst[:, 24:40, :], in_=v_r[:, 24:40, :])
            nc.sync.dma_start(out=kst[:, 40:64, :], in_=k_r[:, 40:64, :])
            nc.sync.dma_start(out=vst[:, 40:64, :], in_=v_r[:, 40:64, :])

            qb = persist.tile([128, 8, 128], BF16)
            nc.vector.tensor_copy(out=qb, in_=qst)
            nc.scalar.dma_start_transpose(out=qt, in_=qb)
            nc.gpsimd.memset(acc6, 0.0)

            NKG = KT_TILES // 4
            NVS = KT_TILES // 8
            vsbs = [
                persist.tile([128, 8, 128], BF16, name=f"vsb{s}")
                for s in range(NVS)
            ]
            ktgs, pts = {}, {}
            k_transposed = [0]  # next K group to cast + transpose
            v_cast = [0]        # next V stage to cast

            def ensure_k(upto):  # make ktg groups [0, upto) available
                while k_transposed[0] < min(upto, NKG):
                    g = k_transposed[0]
                    kb = kb_pool.tile([128, 4, 128], BF16, tag="kb")
                    nc.vector.tensor_copy(
                        out=kb, in_=kst[:, 4 * g : 4 * g + 4, :]
                    )
                    ktg = ktg_pool.tile([128, 4, 128], BF16, tag="ktg")
                    nc.scalar.dma_start_transpose(out=ktg, in_=kb)
                    ktgs[g] = ktg
                    k_transposed[0] += 1

            def ensure_v(upto):  # make vsb stages [0, upto) available
                while v_cast[0] < min(upto, NVS):
                    s = v_cast[0]
                    nc.gpsimd.tensor_copy(
                        out=vsbs[s], in_=vst[:, 8 * s : 8 * s + 8, :]
                    )
                    v_cast[0] += 1

            # prime the pipelines
            ensure_k(2)
            ensure_v(2)

            # S^T ring: 6 banks of [128, 512] chunks; chunk (2t+c) % 6
            sring = psum_s.tile([128, 6, 512], F32)
            sflat = sring.rearrange("p a f -> p (a f)")  # [128, 3072]
            po = psum_o.tile([128, NQS], F32)  # O^T accum, both chunks

            s_chunk = [0]   # next S^T chunk to produce (2t+c)

            def produce_chunks(upto):  # S^T matmuls for chunks [0, upto)
                while s_chunk[0] < min(upto, NCHUNKS):
                    k = s_chunk[0]
                    t, c = divmod(k, 2)
                    g4, a = divmod(t, 4)
                    ensure_k(g4 + 3)
                    nc.tensor.matmul(
                        sring[:, k % 6, :],
                        ktgs[g4][:, a, :],
                        qt[:, 4 * c : 4 * c + 4, :],
                        start=True,
                        stop=True,
                    )
                    if a == 3 and c == 1:
                        ktgs.pop(g4)
                    s_chunk[0] += 1

            def exp_window(w):  # exp over chunks 3w..3w+2, l-accum
                start, n = WINDOWS[w]
                lo = (start % 6) * 512
                pt = pt_pool.tile([128, 3, 512], BF16, tag="pt")
                ptf = pt.rearrange("p i f -> p (i f)")
                nc.scalar.activation(
                    ptf[:, : 512 * n], sflat[:, lo : lo + 512 * n], EXP,
                    scale=float(SCALE),
                )
                nc.vector.tensor_add(
                    acc6f[:, lo : lo + 512 * n],
                    acc6f[:, lo : lo + 512 * n],
                    ptf[:, : 512 * n],
                )
                pts[w] = pt

            pv_chunk = [0]  # next chunk to consume in PV

            def pv_consume(upto):  # PV matmuls for chunks [0, upto)
                while pv_chunk[0] < min(upto, NCHUNKS):
                    k = pv_chunk[0]
                    t, c = divmod(k, 2)
                    ensure_v(t // 8 + 2)
                    w, i = divmod(k, 3)
                    nc.tensor.matmul(
                        po[:, 512 * c : 512 * (c + 1)],
                        vsbs[t // 8][:, t % 8, :],
                        pts[w][:, i, :],
                        start=(t == 0),
                        stop=(t == KT_TILES - 1),
                    )
                    if i == 2 or k == NCHUNKS - 1:
                        pts.pop(w - 1, None)  # w-1 fully consumed earlier
                    pv_chunk[0] += 1

            # --- main pipeline. Emission order per iteration is
            # [S(w), exp(w-1), PV(w-2)]: S runs one window ahead of exp
            # so that by the time exp(w-1) completes, exp(w)'s input
            # matmuls already retired - the PE's in-order completion
            # counter then never makes an exp wait on a PV that itself
            # waits on the previous exp (which would serialize
            # exp -> PV -> exp). PV trails two windows.
            NW = len(WINDOWS)
            for w in range(NW + 2):
                if w < NW:
                    produce_chunks(WINDOWS[w][0] + WINDOWS[w][1])
                if 1 <= w <= NW:
                    exp_window(w - 1)
                if w >= 2:
                    pv_consume(WINDOWS[w - 2][0] + WINDOWS[w - 2][1])

            # --- epilogue ---
            # l: fold 6 lanes -> [q, a] via XBAR transpose + reduces.
            # acct[qf, g, kf] = acc6f[kf, 128g + qf]; lane L = g//4 holds
            # q-half L%2, sub-block a4 = g%4.
            acct = persist.tile([128, 24, 128], BF16)
            nc.scalar.dma_start_transpose(out=acct, in_=acc6f)
            r24 = persist.tile([128, 24], F32)
            nc.vector.tensor_reduce(
                r24, acct, axis=mybir.AxisListType.X, op=mybir.AluOpType.add
            )
            r24v = r24.rearrange("p (L a) -> p L a", L=6)
            for c in range(2):
                nc.vector.tensor_reduce(
                    lq[:, 4 * c : 4 * c + 4],
                    r24v[:, c::2, :].rearrange("p L a -> p a L"),
                    axis=mybir.AxisListType.X,
                    op=mybir.AluOpType.add,
                )
            nc.vector.reciprocal(rlq, lq)
            # O: cast O^T to bf16, transpose, scale rows by 1/l
            nc.vector.tensor_copy(out=ob, in_=po)
            nc.sync.dma_start_transpose(out=ot, in_=ob)
            for a in range(8):
                nc.vector.tensor_scalar_mul(
                    out_sb[:, a, :], ot[:, a, :], rlq[:, a : a + 1]
                )
            nc.sync.dma_start(out=o_r, in_=out_sb)

    nc.compile()
    return nc


def _get_compiled():
    global _COMPILED
    if _COMPILED is None:
        _COMPILED = _build()
    return _COMPILED


def kernel(Q, K, V):
    assert Q.shape == (NQ, D) and K.shape == (NK, D) and V.shape == (NK, D), (
        Q.shape, K.shape, V.shape
    )
    Q = np.ascontiguousarray(np.asarray(Q, dtype=np.float32))
    K = np.ascontiguousarray(np.asarray(K, dtype=np.float32))
    V = np.ascontiguousarray(np.asarray(V, dtype=np.float32))
    nc = _get_compiled()
    in_maps = [
        {"Q": Q[i * NQS : (i + 1) * NQS], "K": K, "V": V} for i in range(N_CORES)
    ]
    res = run_bass_kernel_spmd(nc, in_maps, list(range(N_CORES)))
    out = np.concatenate([r["out"] for r in res.results], axis=0)
    return out.astype(np.float32)
